# revision 1
# baseline (speedup 1.0000x reference)
"""NonLocalAttention (embedded gaussian, no softmax) on 8 trn2 NeuronCores.

Reference math (per sample, all linear — no softmax):
    theta = conv1x1(a, theta_w, theta_b)        # [Ci, N]
    phi   = conv1x1(b, phi_w, phi_b)            # [Ci, N]
    g     = conv1x1(b, g_w, g_b)                # [Ci, N]
    f     = theta^T @ phi / N                   # [N, N]
    y     = f @ g^T                             # [N, Ci]
    out   = BN(W_w @ y^T)                       # [C, N]

Associativity rewrite: there is no nonlinearity between the two big
matmuls, so the NxN attention map need never be materialized:
    Mi[ci1, ci2] = sum_m phi[ci1, m] * g[ci2, m]          # [128, 128]
    y^T[ci2, n]  = sum_ci1 Mi[ci1, ci2] * theta[ci1, n]   # (1/N in theta)
Per-core compute drops from ~2.1 GMAC to ~0.13 GMAC; the kernel runs near
the memory roofline.

Sharding: 8 cores = 4 samples x 2 pixel-halves. Core (s, h) computes output
pixels [h*2048, (h+1)*2048) of sample s: theta conv on its half of a, phi/g
convs + Mi on the full b (duplicated across the 2 cores of a sample, cheap),
zero inter-core communication.

The Mi contraction runs over pixels, which needs phi/g tiles with pixels on
partitions: produced by PE transpose-mode (matmul vs identity) on 128x128
tiles of the conv outputs. Transpose evictions alternate DVE / ACT engines.

DMAs are batched coarsely (each dma_start costs ~0.6us of serial issue) and
ordered consts -> a -> b so the theta conv starts while b still streams.

PRECISION = "f32" (default): all matmuls in true fp32 (PE LOW+HIGH two-pass
mode), output matches the jax reference to ~7e-7 relative.
PRECISION = "f32r": TF32-like single-pass PE mode, 4x faster matmuls,
~3e-4 relative error end-to-end (measured on HW).
"""

import numpy as np

B, C, Ci, H, W = 4, 256, 128, 64, 64
N_PIX = H * W            # 4096 pixels per sample
N_CORES = 8
HALF = N_PIX // 2        # 2048 output pixels per core
P = 128
CC = C // P              # 2 channel chunks
RB = 512                 # row block (max 4-byte moving free dim)
MCH = N_PIX // P         # 32 pixel chunks for the Mi contraction
BN_EPS = 1e-5

PRECISION = "f32r"       # "f32r" | "f32"

_CACHE = {}


def _build(precision=PRECISION):
    import concourse.bacc as bacc
    import concourse.mybir as mybir
    import concourse.tile as tile
    from concourse.masks import make_identity

    f32 = mybir.dt.float32
    fmm = mybir.dt.float32r if precision == "f32r" else f32
    Act = mybir.ActivationFunctionType

    # Bacc (not raw Bass): compile() legalizes sync waits (TRN2 allows at
    # most one sem wait per instruction; excess waits split onto
    # InstEventSemaphore / moved to ldweights).
    nc = bacc.Bacc("TRN2", num_devices=N_CORES)

    # packed weights: [thetaT(2x128) | phiT(2x128) | gwT(2x128) | WT(256)]
    wpack_d = nc.dram_tensor("wpack", [P, 4 * C], fmm, kind="ExternalInput")
    # packed f32 per-partition vectors:
    # [tb | pb | gb | scale cc0 | scale cc1 | shift cc0 | shift cc1]
    vpack_d = nc.dram_tensor("vpack", [P, 7], f32, kind="ExternalInput")
    a_d = nc.dram_tensor("a_half", [CC, P, HALF], fmm, kind="ExternalInput")
    b_d = nc.dram_tensor("b_full", [CC, P, N_PIX], fmm, kind="ExternalInput")
    out_d = nc.dram_tensor("out", [CC, P, HALF], f32, kind="ExternalOutput")

    with tile.TileContext(nc) as tc:
        with (
            tc.tile_pool(name="const", bufs=1) as cpool,
            tc.tile_pool(name="big", bufs=1) as bpool,
            tc.tile_pool(name="work", bufs=3) as wpool,
            tc.tile_pool(name="ps", bufs=2, space="PSUM") as ppool,
        ):
            # ---- constants: 2 DMAs -----------------------------------------
            wpack_sb = cpool.tile([P, 4 * C], fmm)
            vpack_sb = cpool.tile([P, 7], f32)
            nc.sync.dma_start(out=wpack_sb[:], in_=wpack_d[:])
            nc.sync.dma_start(out=vpack_sb[:], in_=vpack_d[:])
            thetaT_sb = wpack_sb[:, 0:C].rearrange("p (c k) -> p c k", c=CC)
            phiT_sb = wpack_sb[:, C : 2 * C].rearrange("p (c k) -> p c k", c=CC)
            gwT_sb = wpack_sb[:, 2 * C : 3 * C].rearrange("p (c k) -> p c k", c=CC)
            WT_sb = wpack_sb[:, 3 * C : 4 * C]
            tb_sb, pb_sb, gb_sb = (vpack_sb[:, i : i + 1] for i in range(3))
            scale_sb = vpack_sb[:, 3:5]
            shift_sb = vpack_sb[:, 5:7]

            # gpsimd memset/affine_select reject f32r and the BIR verifier
            # wants f32r matmul operands produced as f32r: build the identity
            # in f32, then round it into the matmul dtype with a DVE copy
            ident_f32 = cpool.tile([P, P], f32)
            ident_sb = cpool.tile([P, P], fmm)
            make_identity(nc, ident_f32[:])
            nc.vector.tensor_copy(ident_sb[:], ident_f32[:])

            # ---- activation loads: a first (theta), then b halves ----------
            a_sb = bpool.tile([P, CC, HALF], fmm)
            b_sb = bpool.tile([P, CC, N_PIX], fmm)
            for hh in range(2):
                s = hh * (HALF // 2)
                for cc in range(CC):
                    nc.sync.dma_start(
                        out=a_sb[:, cc, s : s + HALF // 2],
                        in_=a_d[cc, :, s : s + HALF // 2],
                    )
            for hh in range(2):
                s = hh * (N_PIX // 2)
                for cc in range(CC):
                    nc.sync.dma_start(
                        out=b_sb[:, cc, s : s + N_PIX // 2],
                        in_=b_d[cc, :, s : s + N_PIX // 2],
                    )

            # ---- theta conv: theta_x[Ci, HALF] (1/N + bias folded in) ------
            theta_x = bpool.tile([Ci, HALF], fmm)
            for p in range(HALF // RB):
                th_ps = ppool.tile([P, RB], f32, tag="conv", bufs=2, name="th_ps")
                for cc in range(CC):
                    nc.tensor.matmul(
                        th_ps[:],
                        thetaT_sb[:, cc, :],
                        a_sb[:, cc, p * RB : (p + 1) * RB],
                        start=(cc == 0),
                        stop=(cc == CC - 1),
                    )
                nc.scalar.activation(
                    theta_x[:, p * RB : (p + 1) * RB], th_ps[:], Act.Identity,
                    bias=tb_sb,
                )

            # ---- phi/g convs + transpose + Mi accumulation, per quarter ----
            phi_x = bpool.tile([Ci, N_PIX], fmm)
            g_x = bpool.tile([Ci, N_PIX], fmm)
            mi_ps = ppool.tile([Ci, Ci], f32, tag="mi", bufs=1, name="mi_ps")
            for q in range(4):
                for p in (2 * q, 2 * q + 1):
                    sl = slice(p * RB, (p + 1) * RB)
                    ph_ps = ppool.tile([P, RB], f32, tag="conv", bufs=2, name="ph_ps")
                    for cc in range(CC):
                        nc.tensor.matmul(
                            ph_ps[:],
                            phiT_sb[:, cc, :],
                            b_sb[:, cc, sl],
                            start=(cc == 0),
                            stop=(cc == CC - 1),
                        )
                    nc.scalar.activation(
                        phi_x[:, sl], ph_ps[:], Act.Identity, bias=pb_sb,
                    )
                    g_ps = ppool.tile([P, RB], f32, tag="conv", bufs=2, name="g_ps")
                    for cc in range(CC):
                        nc.tensor.matmul(
                            g_ps[:],
                            gwT_sb[:, cc, :],
                            b_sb[:, cc, sl],
                            start=(cc == 0),
                            stop=(cc == CC - 1),
                        )
                    nc.scalar.activation(
                        g_x[:, sl], g_ps[:], Act.Identity, bias=gb_sb,
                    )
                for m in range(8 * q, 8 * q + 8):
                    sl = slice(m * P, (m + 1) * P)
                    tpp_ps = ppool.tile([P, P], fmm, tag="tp", bufs=3, name="tpp_ps")
                    nc.tensor.transpose(tpp_ps[:], phi_x[:, sl], ident_sb[:])
                    phiT_m = wpool.tile([P, Ci], fmm, tag="phiT_m", bufs=3,
                                        name="phiT_m")
                    nc.vector.tensor_copy(phiT_m[:], tpp_ps[:])
                    tpg_ps = ppool.tile([P, P], fmm, tag="tp", bufs=3, name="tpg_ps")
                    nc.tensor.transpose(tpg_ps[:], g_x[:, sl], ident_sb[:])
                    gT_m = wpool.tile([P, Ci], fmm, tag="gT_m", bufs=3, name="gT_m")
                    nc.scalar.activation(gT_m[:], tpg_ps[:], Act.Copy)
                    nc.tensor.matmul(
                        mi_ps[:], phiT_m[:], gT_m[:],
                        start=(m == 0), stop=(m == MCH - 1),
                    )
            mi_sb = wpool.tile([Ci, Ci], fmm, tag="mi_sb", bufs=1, name="mi_sb")
            nc.vector.tensor_copy(mi_sb[:], mi_ps[:])

            # ---- y^T = Mi^T-contract theta_x; W conv; BN; store ------------
            for r in range(HALF // RB):
                rows = slice(r * RB, (r + 1) * RB)
                yt_ps = ppool.tile([Ci, RB], f32, tag="yt", bufs=2, name="yt_ps")
                nc.tensor.matmul(
                    yt_ps[:], mi_sb[:], theta_x[:, rows], start=True, stop=True,
                )
                yT_sb = wpool.tile([Ci, RB], fmm, tag="ysb", bufs=2, name="yT_sb")
                nc.vector.tensor_copy(yT_sb[:], yt_ps[:])
                osb = wpool.tile([P, CC, RB], f32, tag="osb", bufs=2, name="osb")
                for cc in range(CC):
                    wy_ps = ppool.tile([P, RB], f32, tag="conv", bufs=2, name="wy_ps")
                    nc.tensor.matmul(
                        wy_ps[:],
                        WT_sb[:, cc * P : (cc + 1) * P],
                        yT_sb[:],
                        start=True,
                        stop=True,
                    )
                    nc.scalar.activation(
                        osb[:, cc, :], wy_ps[:], Act.Identity,
                        bias=shift_sb[:, cc : cc + 1],
                        scale=scale_sb[:, cc : cc + 1],
                    )
                nc.sync.dma_start(
                    out=out_d[:, :, rows].rearrange("c p r -> p c r"), in_=osb[:],
                )

    nc.compile()
    return nc


def _get_nc():
    if "nc" not in _CACHE:
        _CACHE["nc"] = _build()
    return _CACHE["nc"]


def _prep_in_maps(a, b, theta_w, theta_b, phi_w, phi_b, g_w, g_b, W_w,
                  bn_gamma, bn_beta, bn_mean, bn_var):
    f = np.float32
    a4 = np.ascontiguousarray(np.asarray(a, f).reshape(B, C, N_PIX))
    b4 = np.ascontiguousarray(np.asarray(b, f).reshape(B, C, N_PIX))

    inv_n = 1.0 / np.float64(N_PIX)
    thetaT = (np.asarray(theta_w, f).T * inv_n).astype(f)   # [C, Ci]
    phiT = np.asarray(phi_w, f).T                           # [C, Ci]
    gwT = np.asarray(g_w, f).T                              # [C, Ci]
    WT = np.asarray(W_w, f).T                               # [Ci, C]
    # wpack rows: partition p; cols: [thetaT cc0|cc1 | phiT cc0|cc1 |
    #                                 gwT cc0|cc1 | WT]
    wpack = np.empty((P, 4 * C), f)
    for i, wT in enumerate((thetaT, phiT, gwT)):
        for cc in range(CC):
            wpack[:, i * C + cc * Ci : i * C + (cc + 1) * Ci] = \
                wT[cc * P : (cc + 1) * P, :]
    wpack[:, 3 * C : 4 * C] = WT
    wpack = np.ascontiguousarray(wpack)

    scale = (np.asarray(bn_gamma, f) / np.sqrt(np.asarray(bn_var, f) + BN_EPS)).astype(f)
    shift = (np.asarray(bn_beta, f) - np.asarray(bn_mean, f) * scale).astype(f)
    vpack = np.stack([
        (np.asarray(theta_b, f) * inv_n).astype(f),
        np.asarray(phi_b, f),
        np.asarray(g_b, f),
        scale[:P], scale[P:],
        shift[:P], shift[P:],
    ], axis=1)
    vpack = np.ascontiguousarray(vpack)

    in_maps = []
    for core in range(N_CORES):
        s, h = divmod(core, 2)
        in_maps.append({
            "a_half": np.ascontiguousarray(
                a4[s][:, h * HALF : (h + 1) * HALF].reshape(CC, P, HALF)),
            "b_full": np.ascontiguousarray(b4[s].reshape(CC, P, N_PIX)),
            "wpack": wpack,
            "vpack": vpack,
        })
    return in_maps


def run(inputs: dict, trace: bool = False):
    from concourse.bass_utils import run_bass_kernel_spmd

    nc = _get_nc()
    in_maps = _prep_in_maps(**inputs)
    res = run_bass_kernel_spmd(nc, in_maps, list(range(N_CORES)), trace=trace)
    out = np.empty((B, C, N_PIX), np.float32)
    for core in range(N_CORES):
        s, h = divmod(core, 2)
        out[s][:, h * HALF : (h + 1) * HALF] = res.results[core]["out"].reshape(C, HALF)
    return out.reshape(B, C, H, W), res


def kernel(**inputs) -> np.ndarray:
    out, _ = run(inputs, trace=False)
    return out



# revision 2
# speedup vs baseline: 1.9223x; 1.9223x over previous
"""NonLocalAttention (embedded gaussian, no softmax) on 8 trn2 NeuronCores.

Reference math (per sample, all linear — no softmax):
    theta = conv1x1(a, theta_w, theta_b)        # [Ci, N]
    phi   = conv1x1(b, phi_w, phi_b)            # [Ci, N]
    g     = conv1x1(b, g_w, g_b)                # [Ci, N]
    f     = theta^T @ phi / N                   # [N, N]
    y     = f @ g^T                             # [N, Ci]
    out   = BN(W_w @ y^T)                       # [C, N]

Everything is linear, so the whole network collapses to a per-sample
256x256 channel-mixing matrix applied to `a`:
    Mi[ci1, ci2] = sum_m phi[ci1, m] * g[ci2, m]          # [128, 128]
    G^T          = (Mi^T @ theta_w / N)^T-ish:  built as
                   Q  = Mi^T theta_w / N        # [128, 256]
                   G^T = Q^T-contract W^T       # [256(cin), 256(cout)]
    out          = G^T-contract a  (+ BN shift)           # [256, N]

Mi is produced without PE transposes: for each 128-pixel chunk of b,
matmul(lhsT=b_chunk, rhs=w^T) directly yields phi^T/g^T tiles with
pixels on partitions, which feed the Mi contraction.

All activations and weights move to the device as bf16 (halves HBM
traffic); accumulation stays f32 in PSUM; output returns as bf16 and is
cast to f32 on the host. Biases (zero in this problem, but handled
exactly): phi_b/g_b fold into a host-computed rank-2 correction to Mi
(needs only rowsums of b); theta_b folds into a per-channel shift via
two tiny on-device matmuls; BN scale folds into W^T on the host.

Sharding: 8 cores = 4 samples x 2 pixel-halves of `a`. Each core loads
the full per-sample b (Mi is duplicated across the pair — cheaper than
any cross-core exchange) and its half of a; no inter-core communication.
"""

import numpy as np

B, C, Ci, H, W = 4, 256, 128, 64, 64
N_PIX = H * W            # 4096 pixels per sample
N_CORES = 8
HALF = N_PIX // 2        # 2048 output pixels per core
P = 128
CC = C // P              # 2 channel chunks
NQB = 4                  # b DMA chunks (1024 px each)
NQUAD = 8                # phase-1 quads (4 pixel-chunks of 128 = 512 px)
RB = 512                 # output row block
BN_EPS = 1e-5

WARMUP_MM = 4            # junk matmuls to lift the PE HAM throttle early

# wpack column layout (bf16, partition dim = 128):
#   [0,256)    phiT   per cc: [c_in_chunk, ci]
#   [256,512)  gT     per cc: [c_in_chunk, ci]
#   [512,768)  theta_w / N   [ci1, c]
#   [768,1024) (W_w * bn_scale)^T  [ci2, c_out]
#   [1024]     theta_b / N   [ci1]
WCOLS = 1025
# vpack (f32): [0,2) bn shift per cc, [2,130) host Mi bias correction
VCOLS = 130

_CACHE = {}


def _build():
    import concourse.bacc as bacc
    import concourse.mybir as mybir
    import concourse.tile as tile

    f32 = mybir.dt.float32
    bf16 = mybir.dt.bfloat16
    Act = mybir.ActivationFunctionType

    nc = bacc.Bacc("TRN2", num_devices=N_CORES)

    wpack_d = nc.dram_tensor("wpack", [P, WCOLS], bf16, kind="ExternalInput")
    vpack_d = nc.dram_tensor("vpack", [P, VCOLS], f32, kind="ExternalInput")
    a_d = nc.dram_tensor("a_half", [CC, P, HALF], bf16, kind="ExternalInput")
    b_d = nc.dram_tensor("b_full", [NQB, CC, P, N_PIX // NQB], bf16,
                         kind="ExternalInput")
    out_d = nc.dram_tensor("out", [CC, P, HALF], bf16, kind="ExternalOutput")

    with tile.TileContext(nc) as tc:
        with (
            tc.tile_pool(name="const", bufs=1) as cpool,
            tc.tile_pool(name="big", bufs=1) as bpool,
            tc.tile_pool(name="work", bufs=2) as wpool,
            tc.tile_pool(name="ps", bufs=6, space="PSUM") as ppool,
        ):
            wpack_sb = cpool.tile([P, WCOLS], bf16)
            vpack_sb = cpool.tile([P, VCOLS], f32)
            a_sb = bpool.tile([P, CC, HALF], bf16)
            b_sb = bpool.tile([P, CC, N_PIX], bf16)

            phiT_w = wpack_sb[:, 0:256].rearrange("p (c k) -> p c k", c=CC)
            gT_w = wpack_sb[:, 256:512].rearrange("p (c k) -> p c k", c=CC)
            thw_sb = wpack_sb[:, 512:768]
            WT_sb = wpack_sb[:, 768:1024]
            thb_sb = wpack_sb[:, 1024:1025]
            shift_in = vpack_sb[:, 0:2]
            cmi_sb = vpack_sb[:, 2:130]

            # weights first (convs + warmup need them), then b chunks
            # (phase 1 streams them), then the phase-2/3 consts and a.
            nc.sync.dma_start(out=wpack_sb[:, 0:512], in_=wpack_d[:, 0:512])
            qp = N_PIX // NQB
            for q in range(NQB):
                nc.sync.dma_start(
                    out=b_sb[:, :, q * qp : (q + 1) * qp],
                    in_=b_d[q].rearrange("c p x -> p c x"),
                )
            nc.sync.dma_start(out=a_sb[:], in_=a_d.rearrange("c p x -> p c x"))
            nc.scalar.dma_start(out=wpack_sb[:, 512:WCOLS],
                                in_=wpack_d[:, 512:WCOLS])
            nc.scalar.dma_start(out=vpack_sb[:], in_=vpack_d[:])

            # ---- PE warmup: garbage matmuls on the weight pack ------------
            if WARMUP_MM:
                warm_ps = ppool.tile([P, RB], f32, tag="ps", name="warm_ps")
                for i in range(WARMUP_MM):
                    nc.tensor.matmul(
                        warm_ps[:], wpack_sb[:, 0:P], wpack_sb[:, 0:RB],
                        start=True, stop=True,
                    )

            # ---- phase 1: Mi accumulation, software-pipelined by quad -----
            # quad qd covers pixel chunks 4qd..4qd+3 (128 px each).
            mi_ps = ppool.tile([Ci, Ci], f32, tag="mi", bufs=1, name="mi_ps")
            ph_sbs, g_sbs = {}, {}

            def emit_quad(qd):
                ph_ps = ppool.tile([P, 512], f32, tag="ps", name=f"phps{qd}")
                g_ps = ppool.tile([P, 512], f32, tag="ps", name=f"gps{qd}")
                for k in range(4):
                    m = 4 * qd + k
                    sl = slice(k * P, (k + 1) * P)
                    for cc in range(CC):
                        bT = b_sb[:, cc, m * P : (m + 1) * P]
                        nc.tensor.matmul(ph_ps[:, sl], bT, phiT_w[:, cc, :],
                                         start=(cc == 0), stop=(cc == CC - 1))
                        nc.tensor.matmul(g_ps[:, sl], bT, gT_w[:, cc, :],
                                         start=(cc == 0), stop=(cc == CC - 1))
                ph_sb = wpool.tile([P, 512], bf16, tag="phsb", name=f"phsb{qd}")
                g_sb = wpool.tile([P, 512], bf16, tag="gsb", name=f"gsb{qd}")
                nc.vector.tensor_copy(ph_sb[:], ph_ps[:])
                nc.scalar.copy(g_sb[:], g_ps[:])
                ph_sbs[qd], g_sbs[qd] = ph_sb, g_sb

            def emit_mi(qd):
                for k in range(4):
                    sl = slice(k * P, (k + 1) * P)
                    nc.tensor.matmul(
                        mi_ps[:], ph_sbs[qd][:, sl], g_sbs[qd][:, sl],
                        start=(qd == 0 and k == 0),
                        stop=(qd == NQUAD - 1 and k == 3),
                    )

            emit_quad(0)
            for qd in range(1, NQUAD):
                emit_quad(qd)
                emit_mi(qd - 1)
            emit_mi(NQUAD - 1)

            # ---- phase 2: Mi -> Q -> G^T (+ bias shift), all tiny ---------
            mi_sb = bpool.tile([Ci, Ci], bf16)
            nc.vector.tensor_tensor(mi_sb[:], mi_ps[:], cmi_sb,
                                    op=mybir.AluOpType.add)

            q_ps = ppool.tile([Ci, C], f32, tag="ps", name="q_ps")
            nc.tensor.matmul(q_ps[:], mi_sb[:], thw_sb, start=True, stop=True)
            q_sb = bpool.tile([Ci, C], bf16)
            nc.vector.tensor_copy(q_sb[:], q_ps[:])

            # theta_b path: v = Mi^T theta_b/N; sh = shift + W^T' v
            v_ps = ppool.tile([P, 4], f32, tag="ps", name="v_ps")
            nc.tensor.matmul(v_ps[:, 0:1], mi_sb[:], thb_sb, start=True,
                             stop=True)
            v_sb = bpool.tile([P, 1], bf16)
            nc.vector.tensor_copy(v_sb[:], v_ps[:, 0:1])
            for co in range(CC):
                nc.tensor.matmul(v_ps[:, 1 + co : 2 + co],
                                 WT_sb[:, co * P : (co + 1) * P], v_sb[:],
                                 start=True, stop=True)
            sh_sb = bpool.tile([P, CC], f32)
            nc.vector.tensor_tensor(sh_sb[:], v_ps[:, 1:3], shift_in,
                                    op=mybir.AluOpType.add)

            gt_sb = bpool.tile([P, CC, C], bf16)
            for ci in range(CC):
                gt_ps = ppool.tile([Ci, C], f32, tag="ps", name=f"gtps{ci}")
                nc.tensor.matmul(gt_ps[:], q_sb[:, ci * P : (ci + 1) * P],
                                 WT_sb[:], start=True, stop=True)
                nc.vector.tensor_copy(gt_sb[:, ci, :], gt_ps[:])

            # ---- phase 3: out = G^T-contract a, BN shift, store -----------
            for r in range(HALF // RB):
                rows = slice(r * RB, (r + 1) * RB)
                osb = wpool.tile([P, CC, RB], bf16, tag="osb", name=f"osb{r}")
                for co in range(CC):
                    o_ps = ppool.tile([P, RB], f32, tag="ps", name=f"ops{r}{co}")
                    for ci in range(CC):
                        nc.tensor.matmul(
                            o_ps[:], gt_sb[:, ci, co * P : (co + 1) * P],
                            a_sb[:, ci, rows],
                            start=(ci == 0), stop=(ci == CC - 1),
                        )
                    if co == 0:
                        nc.scalar.activation(osb[:, 0, :], o_ps[:],
                                             Act.Identity,
                                             bias=sh_sb[:, 0:1])
                    else:
                        nc.vector.tensor_tensor(
                            osb[:, 1, :], o_ps[:],
                            sh_sb[:, 1:2].broadcast_to([P, RB]),
                            op=mybir.AluOpType.add,
                        )
                nc.sync.dma_start(
                    out=out_d[:, :, rows].rearrange("c p r -> p c r"),
                    in_=osb[:],
                )

    nc.compile()
    return nc


def _get_nc():
    if "nc" not in _CACHE:
        _CACHE["nc"] = _build()
    return _CACHE["nc"]


def _prep_in_maps(a, b, theta_w, theta_b, phi_w, phi_b, g_w, g_b, W_w,
                  bn_gamma, bn_beta, bn_mean, bn_var):
    import ml_dtypes

    f = np.float32
    bf = ml_dtypes.bfloat16
    a4 = np.asarray(a, f).reshape(B, C, N_PIX)
    b4 = np.asarray(b, f).reshape(B, C, N_PIX)
    theta_w = np.asarray(theta_w, f)
    phi_w = np.asarray(phi_w, f)
    g_w = np.asarray(g_w, f)
    W_w = np.asarray(W_w, f)
    theta_b = np.asarray(theta_b, f)
    phi_b = np.asarray(phi_b, f)
    g_b = np.asarray(g_b, f)

    scale = (np.asarray(bn_gamma, f)
             / np.sqrt(np.asarray(bn_var, f) + BN_EPS)).astype(f)
    shift = (np.asarray(bn_beta, f) - np.asarray(bn_mean, f) * scale).astype(f)
    inv_n = 1.0 / np.float64(N_PIX)

    wpack = np.zeros((P, WCOLS), f)
    wpack[:, 0:128] = phi_w.T[0:P]
    wpack[:, 128:256] = phi_w.T[P:C]
    wpack[:, 256:384] = g_w.T[0:P]
    wpack[:, 384:512] = g_w.T[P:C]
    wpack[:, 512:768] = theta_w * inv_n
    wpack[:, 768:1024] = (W_w * scale[:, None]).T
    wpack[:, 1024] = theta_b * inv_n
    wpack = np.ascontiguousarray(wpack.astype(bf))

    # Mi bias correction from rowsums of b (exact; zero when biases are zero)
    rsb = b4.sum(axis=2)                        # [B, C]
    s_phi = rsb @ phi_w.T                       # [B, Ci]
    s_g = rsb @ g_w.T                           # [B, Ci]
    qp = N_PIX // NQB

    in_maps = []
    for core in range(N_CORES):
        s, h = divmod(core, 2)
        cmi = (phi_b[:, None] * s_g[s][None, :]
               + s_phi[s][:, None] * g_b[None, :]
               + N_PIX * phi_b[:, None] * g_b[None, :]).astype(f)
        vpack = np.zeros((P, VCOLS), f)
        vpack[:, 0] = shift[:P]
        vpack[:, 1] = shift[P:]
        vpack[:, 2:130] = cmi
        in_maps.append({
            "a_half": np.ascontiguousarray(
                a4[s][:, h * HALF : (h + 1) * HALF]
                .reshape(CC, P, HALF).astype(bf)),
            "b_full": np.ascontiguousarray(
                b4[s].reshape(CC, P, NQB, qp)
                .transpose(2, 0, 1, 3).astype(bf)),
            "wpack": wpack,
            "vpack": np.ascontiguousarray(vpack),
        })
    return in_maps


def run(inputs: dict, trace: bool = False):
    from concourse.bass_utils import run_bass_kernel_spmd

    nc = _get_nc()
    in_maps = _prep_in_maps(**inputs)
    res = run_bass_kernel_spmd(nc, in_maps, list(range(N_CORES)), trace=trace)
    out = np.empty((B, C, N_PIX), np.float32)
    for core in range(N_CORES):
        s, h = divmod(core, 2)
        out[s][:, h * HALF : (h + 1) * HALF] = \
            res.results[core]["out"].reshape(C, HALF).astype(np.float32)
    return out.reshape(B, C, H, W), res


def kernel(**inputs) -> np.ndarray:
    out, _ = run(inputs, trace=False)
    return out


# revision 8
# speedup vs baseline: 2.3396x; 1.2171x over previous
"""NonLocalAttention (embedded gaussian, no softmax) on 8 trn2 NeuronCores.

Reference math (per sample, all linear — no softmax):
    theta = conv1x1(a, theta_w, theta_b)        # [Ci, N]
    phi   = conv1x1(b, phi_w, phi_b)            # [Ci, N]
    g     = conv1x1(b, g_w, g_b)                # [Ci, N]
    f     = theta^T @ phi / N                   # [N, N]
    y     = f @ g^T                             # [N, Ci]
    out   = BN(W_w @ y^T)                       # [C, N]

Everything is linear, so the whole network collapses to a per-sample
256x256 channel-mixing matrix applied to `a`:
    Mi[ci1, ci2] = sum_m phi[ci1, m] * g[ci2, m]          # [128, 128]
    R^T = Mi-contract W'^T  (W' = bn_scale * W_w)         # [128, 256]
    ta  = theta'^T-contract a  (theta' = theta_w/N)       # [128, N]
    out = R^T-contract ta + shift                         # [256, N]

Mi is produced without PE transposes: for each 128-pixel chunk of b,
matmul(lhsT=b_chunk, rhs=[phiT|gT]) directly yields phi^T/g^T tiles with
pixels on partitions, which feed the Mi contraction.

All activations and weights move to the device as bf16 (halves HBM
traffic); accumulation stays f32 in PSUM; output returns as bf16 and is
cast to f32 on the host. Biases (zero in this problem, but handled
exactly): phi_b/g_b fold into a host-computed rank-2 correction to Mi
(needs only rowsums of b); theta_b is the bias of the ta eviction; BN
scale folds into W^T on the host.

Sharding: 8 cores = 4 samples x 2 pixel-halves of `a`. Each core loads
the full per-sample b (Mi is duplicated across the pair — cheaper than
any cross-core exchange) and its half of a; no inter-core communication.
"""

import numpy as np

B, C, Ci, H, W = 4, 256, 128, 64, 64
N_PIX = H * W            # 4096 pixels per sample
N_CORES = 8
HALF = N_PIX // 2        # 2048 output pixels per core
P = 128
CC = C // P              # 2 channel chunks
NCH = 8                  # b DMA chunks (512 px each) == phase-1 quads
QPIX = N_PIX // NCH      # 512 pixels per chunk/quad
RB = 512                 # output row block
BN_EPS = 1e-5

WARMUP_MM = 6            # junk matmuls to lift the PE HAM throttle early

# wpack column layout (bf16, partition dim = 128):
#   [0,256)     cc0: [phiT | gT]    [c_in_chunk, ci]
#   [256,512)   cc1: [phiT | gT]
#   [512,768)   (theta_w/N)^T      [c (2 chunks), ci1]
#   [768,1024)  (W_w * bn_scale)^T [ci2, c_out]
WCOLS = 1024
# vpack (f32): [0,2) bn shift per cc, [2] theta_b/N, [3,131) Mi correction
VCOLS = 131

_CACHE = {}


def _build():
    import concourse.bacc as bacc
    import concourse.mybir as mybir
    import concourse.tile as tile

    f32 = mybir.dt.float32
    bf16 = mybir.dt.bfloat16
    Act = mybir.ActivationFunctionType

    nc = bacc.Bacc("TRN2", num_devices=N_CORES)

    wpack_d = nc.dram_tensor("wpack", [P, WCOLS], bf16, kind="ExternalInput")
    vpack_d = nc.dram_tensor("vpack", [P, VCOLS], f32, kind="ExternalInput")
    a_d = nc.dram_tensor("a_half", [CC, P, HALF], bf16, kind="ExternalInput")
    b_d = nc.dram_tensor("b_full", [NCH, CC, P, QPIX], bf16,
                         kind="ExternalInput")
    out_d = nc.dram_tensor("out", [CC, P, HALF], bf16, kind="ExternalOutput")

    with tile.TileContext(nc) as tc:
        with (
            tc.tile_pool(name="const", bufs=1) as cpool,
            tc.tile_pool(name="big", bufs=1) as bpool,
            tc.tile_pool(name="work", bufs=2) as wpool,
            tc.tile_pool(name="ps", bufs=3, space="PSUM") as ppool,
        ):
            wpack_sb = cpool.tile([P, WCOLS], bf16)
            vpack_sb = cpool.tile([P, VCOLS], f32)
            a_sb = bpool.tile([P, CC, HALF], bf16)
            b_sb = bpool.tile([P, CC, N_PIX], bf16)

            conv_w = wpack_sb[:, 0:512].rearrange("p (c k) -> p c k", c=CC)
            thwT = wpack_sb[:, 512:768].rearrange("p (c k) -> p c k", c=CC)
            WT_sb = wpack_sb[:, 768:1024]
            shift_in = vpack_sb[:, 0:2]
            thb_sb = vpack_sb[:, 2:3]
            cmi_sb = vpack_sb[:, 3:131]

            # single SP FIFO keeps the transfer order exactly as needed:
            # conv weights, b chunks (phase 1 streams them), phase-2/3
            # consts, a (only needed by phase 3), then the output stores.
            nc.sync.dma_start(out=wpack_sb[:, 0:512], in_=wpack_d[:, 0:512])
            for q in range(NCH):
                nc.sync.dma_start(
                    out=b_sb[:, :, q * QPIX : (q + 1) * QPIX],
                    in_=b_d[q].rearrange("c p x -> p c x"),
                )
            nc.sync.dma_start(out=wpack_sb[:, 512:WCOLS],
                              in_=wpack_d[:, 512:WCOLS])
            nc.sync.dma_start(out=vpack_sb[:], in_=vpack_d[:])
            nc.sync.dma_start(out=a_sb[:], in_=a_d.rearrange("c p x -> p c x"))

            # ---- engine warmup ------------------------------------------
            # Touch the scalar engine immediately so its activation-table
            # load (1.3us) runs during the initial DMA wait, not in front of
            # the first phase-1 eviction.
            act_warm = cpool.tile([P, 8], f32)
            nc.scalar.memzero(act_warm[:, 0:4])
            nc.scalar.copy(act_warm[:, 4:8], act_warm[:, 0:4])

            # ---- PE warmup: garbage matmuls on the weight pack ------------
            if WARMUP_MM:
                warm_ps = ppool.tile([P, RB], f32, tag="phi", name="warm_ps")
                for i in range(WARMUP_MM):
                    nc.tensor.matmul(
                        warm_ps[:, 0:256], wpack_sb[:, 0:P],
                        wpack_sb[:, 0:256], start=True, stop=True,
                    )

            # ---- phase 1: Mi accumulation, software-pipelined by quad -----
            # quad qd = pixel chunks 4qd..4qd+3 (128 px each) = b chunk qd.
            mi_ps = ppool.tile([Ci, Ci], f32, tag="mi", bufs=1, name="mi_ps")
            q_sbs = {}

            def emit_quad(qd):
                # [pix, 2 x (phiT | gT)] lo/hi halves; fully separate PSUM
                # tiles so the DVE and ACT evictions share no dependencies.
                lo_ps = ppool.tile([P, 2, 256], f32, tag="plo", name=f"lops{qd}")
                hi_ps = ppool.tile([P, 2, 256], f32, tag="phi", name=f"hips{qd}")
                for k in range(4):
                    m = 4 * qd + k
                    dst = lo_ps if k < 2 else hi_ps
                    for cc in range(CC):
                        nc.tensor.matmul(
                            dst[:, k % 2, :],
                            b_sb[:, cc, m * P : (m + 1) * P],
                            conv_w[:, cc, :],
                            start=(cc == 0), stop=(cc == CC - 1),
                        )
                qd_lo = wpool.tile([P, 2, 256], bf16, tag="qlo", bufs=3,
                                   name=f"qlo{qd}")
                qd_hi = wpool.tile([P, 2, 256], bf16, tag="qhi", bufs=3,
                                   name=f"qhi{qd}")
                nc.vector.tensor_copy(qd_lo[:], lo_ps[:])
                nc.scalar.copy(qd_hi[:], hi_ps[:])
                q_sbs[qd] = (qd_lo, qd_hi)

            # flipped: mi_ps[ci2, ci1] = Mi[ci1, ci2] (g as lhsT, phi as
            # rhs) so R^T comes out of a single matmul later.
            def emit_mi(qd, ks=(0, 1, 2, 3)):
                for k in ks:
                    half = q_sbs[qd][k // 2]
                    kk = k % 2
                    nc.tensor.matmul(
                        mi_ps[:], half[:, kk, P:256], half[:, kk, 0:P],
                        start=(qd == 0 and k == 0),
                        stop=(qd == NCH - 1 and k == 3),
                    )

            emit_quad(0)
            for qd in range(1, NCH):
                emit_quad(qd)
                if qd < NCH - 1:
                    emit_mi(qd - 1)
            emit_mi(NCH - 2)

            # ---- tail: finish Mi while ta = theta'^T a fills the PE -------
            NBLK = HALF // RB
            ta_sb = bpool.tile([Ci, HALF], bf16)
            mi_sb = bpool.tile([Ci, Ci], bf16)
            rt_sb = bpool.tile([Ci, C], bf16)

            def emit_ta(t):
                rows = slice(t * RB, (t + 1) * RB)
                ta_ps = ppool.tile([Ci, RB], f32,
                                   tag=("plo" if t % 2 else "phi"),
                                   name=f"taps{t}")
                for cc in range(CC):
                    nc.tensor.matmul(ta_ps[:], thwT[:, cc, :],
                                     a_sb[:, cc, rows],
                                     start=(cc == 0), stop=(cc == CC - 1))
                if t % 2 == 0:
                    nc.scalar.activation(ta_sb[:, rows], ta_ps[:],
                                         Act.Identity, bias=thb_sb)
                else:
                    nc.vector.tensor_tensor(
                        ta_sb[:, rows], ta_ps[:],
                        thb_sb.broadcast_to([Ci, RB]),
                        op=mybir.AluOpType.add)

            emit_mi(NCH - 1, (0, 1))
            emit_ta(0)
            emit_ta(1)
            emit_mi(NCH - 1, (2, 3))
            nc.vector.tensor_tensor(mi_sb[:], mi_ps[:], cmi_sb,
                                    op=mybir.AluOpType.add)
            emit_ta(2)
            rt_ps = ppool.tile([Ci, C], f32, tag="phi", name="rt_ps")
            nc.tensor.matmul(rt_ps[:], mi_sb[:], WT_sb[:],
                             start=True, stop=True)
            nc.vector.tensor_copy(rt_sb[:], rt_ps[:])
            emit_ta(3)

            # ---- out = R^T-contract ta, BN shift, store -------------------
            for r in range(NBLK):
                rows = slice(r * RB, (r + 1) * RB)
                osb = wpool.tile([P, CC, RB], bf16, tag="osb", bufs=4,
                                 name=f"osb{r}")
                for co in range(CC):
                    o_ps = ppool.tile([P, RB], f32,
                                      tag=("plo" if co else "phi"),
                                      name=f"ops{r}{co}")
                    nc.tensor.matmul(o_ps[:], rt_sb[:, co * P : (co + 1) * P],
                                     ta_sb[:, rows], start=True, stop=True)
                    if co == 0:
                        nc.scalar.activation(osb[:, 0, :], o_ps[:],
                                             Act.Identity,
                                             bias=shift_in[:, 0:1])
                    else:
                        nc.vector.tensor_tensor(
                            osb[:, 1, :], o_ps[:],
                            shift_in[:, 1:2].broadcast_to([P, RB]),
                            op=mybir.AluOpType.add,
                        )
                nc.sync.dma_start(
                    out=out_d[:, :, rows].rearrange("c p r -> p c r"),
                    in_=osb[:],
                )

    nc.compile()
    return nc


def _get_nc():
    if "nc" not in _CACHE:
        _CACHE["nc"] = _build()
    return _CACHE["nc"]


def _prep_in_maps(a, b, theta_w, theta_b, phi_w, phi_b, g_w, g_b, W_w,
                  bn_gamma, bn_beta, bn_mean, bn_var):
    import ml_dtypes

    f = np.float32
    bf = ml_dtypes.bfloat16
    a4 = np.asarray(a, f).reshape(B, C, N_PIX)
    b4 = np.asarray(b, f).reshape(B, C, N_PIX)
    theta_w = np.asarray(theta_w, f)
    phi_w = np.asarray(phi_w, f)
    g_w = np.asarray(g_w, f)
    W_w = np.asarray(W_w, f)
    theta_b = np.asarray(theta_b, f)
    phi_b = np.asarray(phi_b, f)
    g_b = np.asarray(g_b, f)

    scale = (np.asarray(bn_gamma, f)
             / np.sqrt(np.asarray(bn_var, f) + BN_EPS)).astype(f)
    shift = (np.asarray(bn_beta, f) - np.asarray(bn_mean, f) * scale).astype(f)
    inv_n = 1.0 / np.float64(N_PIX)

    wpack = np.zeros((P, WCOLS), f)
    wpack[:, 0:128] = phi_w.T[0:P]
    wpack[:, 128:256] = g_w.T[0:P]
    wpack[:, 256:384] = phi_w.T[P:C]
    wpack[:, 384:512] = g_w.T[P:C]
    thT = (theta_w * inv_n).T                   # [C, Ci]
    wpack[:, 512:640] = thT[0:P]
    wpack[:, 640:768] = thT[P:C]
    wpack[:, 768:1024] = (W_w * scale[:, None]).T
    wpack = np.ascontiguousarray(wpack.astype(bf))

    # Mi bias correction from rowsums of b (exact; zero when biases are zero)
    rsb = b4.sum(axis=2)                        # [B, C]
    s_phi = rsb @ phi_w.T                       # [B, Ci]
    s_g = rsb @ g_w.T                           # [B, Ci]

    in_maps = []
    for core in range(N_CORES):
        s, h = divmod(core, 2)
        cmi = (phi_b[:, None] * s_g[s][None, :]
               + s_phi[s][:, None] * g_b[None, :]
               + N_PIX * phi_b[:, None] * g_b[None, :]).astype(f)
        vpack = np.zeros((P, VCOLS), f)
        vpack[:, 0] = shift[:P]
        vpack[:, 1] = shift[P:]
        vpack[:, 2] = theta_b * inv_n
        vpack[:, 3:131] = cmi.T
        in_maps.append({
            "a_half": np.ascontiguousarray(
                a4[s][:, h * HALF : (h + 1) * HALF]
                .reshape(CC, P, HALF).astype(bf)),
            "b_full": np.ascontiguousarray(
                b4[s].reshape(CC, P, NCH, QPIX)
                .transpose(2, 0, 1, 3).astype(bf)),
            "wpack": wpack,
            "vpack": np.ascontiguousarray(vpack),
        })
    return in_maps


def run(inputs: dict, trace: bool = False):
    from concourse.bass_utils import run_bass_kernel_spmd

    nc = _get_nc()
    in_maps = _prep_in_maps(**inputs)
    res = run_bass_kernel_spmd(nc, in_maps, list(range(N_CORES)), trace=trace)
    out = np.empty((B, C, N_PIX), np.float32)
    for core in range(N_CORES):
        s, h = divmod(core, 2)
        out[s][:, h * HALF : (h + 1) * HALF] = \
            res.results[core]["out"].reshape(C, HALF).astype(np.float32)
    return out.reshape(B, C, H, W), res


def kernel(**inputs) -> np.ndarray:
    out, _ = run(inputs, trace=False)
    return out


# revision 12
# speedup vs baseline: 2.3457x; 1.0026x over previous
"""NonLocalAttention (embedded gaussian, no softmax) on 8 trn2 NeuronCores.

Reference math (per sample, all linear — no softmax):
    theta = conv1x1(a, theta_w, theta_b)        # [Ci, N]
    phi   = conv1x1(b, phi_w, phi_b)            # [Ci, N]
    g     = conv1x1(b, g_w, g_b)                # [Ci, N]
    f     = theta^T @ phi / N                   # [N, N]
    y     = f @ g^T                             # [N, Ci]
    out   = BN(W_w @ y^T)                       # [C, N]

Everything is linear, so the whole network collapses to a per-sample
256x256 channel-mixing matrix applied to `a`:
    Mi[ci1, ci2] = sum_m phi[ci1, m] * g[ci2, m]          # [128, 128]
    R^T = Mi-contract W'^T  (W' = bn_scale * W_w)         # [128, 256]
    ta  = theta'^T-contract a  (theta' = theta_w/N)       # [128, N]
    out = R^T-contract ta + shift                         # [256, N]

Mi is produced without PE transposes: for each 128-pixel chunk of b,
matmul(lhsT=b_chunk, rhs=[phiT|gT]) directly yields phi^T/g^T tiles with
pixels on partitions, which feed the Mi contraction.

All activations and weights move to the device as bf16 (halves HBM
traffic); accumulation stays f32 in PSUM; output returns as bf16 and is
cast to f32 on the host. Biases (zero in this problem, but handled
exactly): phi_b/g_b fold into a host-computed rank-2 correction to Mi
(needs only rowsums of b); theta_b is the bias of the ta eviction; BN
scale folds into W^T on the host.

Sharding: 8 cores = 4 samples x 2 pixel-halves of `a`. Each core loads
the full per-sample b (Mi is duplicated across the pair — cheaper than
any cross-core exchange) and its half of a; no inter-core communication.
"""

import numpy as np

B, C, Ci, H, W = 4, 256, 128, 64, 64
N_PIX = H * W            # 4096 pixels per sample
N_CORES = 8
HALF = N_PIX // 2        # 2048 output pixels per core
P = 128
CC = C // P              # 2 channel chunks
NCH = 8                  # phase-1 quads (512 px each)
QPIX = N_PIX // NCH      # 512 pixels per quad
# b DMA chunk sizes in pixels: small first so the convs start early, then
# large enough to stay ahead of the PE while amortizing per-DMA overhead
B_CHUNKS = (128, 128, 256, 512, 512, 512, 512, 512, 512, 512)
RB = 512                 # output row block
BN_EPS = 1e-5

WARMUP_MM = 6            # junk matmuls to lift the PE HAM throttle early

# wpack column layout (bf16, partition dim = 128):
#   [0,256)     cc0: [phiT | gT]    [c_in_chunk, ci]
#   [256,512)   cc1: [phiT | gT]
#   [512,768)   (theta_w/N)^T      [c (2 chunks), ci1]
#   [768,1024)  (W_w * bn_scale)^T [ci2, c_out]
WCOLS = 1024
# vpack (f32): [0,2) bn shift per cc, [2] theta_b/N, [3,131) Mi correction
VCOLS = 131

_CACHE = {}


def _build():
    import concourse.bacc as bacc
    import concourse.mybir as mybir
    import concourse.tile as tile

    f32 = mybir.dt.float32
    bf16 = mybir.dt.bfloat16
    Act = mybir.ActivationFunctionType

    nc = bacc.Bacc("TRN2", num_devices=N_CORES)

    wpack_d = nc.dram_tensor("wpack", [P, WCOLS], bf16, kind="ExternalInput")
    vpack_d = nc.dram_tensor("vpack", [P, VCOLS], f32, kind="ExternalInput")
    a_d = nc.dram_tensor("a_half", [CC, P, HALF], bf16, kind="ExternalInput")
    b_d = nc.dram_tensor("b_full", [CC * P * N_PIX], bf16,
                         kind="ExternalInput")
    out_d = nc.dram_tensor("out", [CC, P, HALF], bf16, kind="ExternalOutput")

    with tile.TileContext(nc) as tc:
        with (
            tc.tile_pool(name="const", bufs=1) as cpool,
            tc.tile_pool(name="big", bufs=1) as bpool,
            tc.tile_pool(name="work", bufs=2) as wpool,
            tc.tile_pool(name="ps", bufs=3, space="PSUM") as ppool,
        ):
            wpack_sb = cpool.tile([P, WCOLS], bf16)
            vpack_sb = cpool.tile([P, VCOLS], f32)
            a_sb = bpool.tile([P, CC, HALF], bf16)
            b_sb = bpool.tile([P, CC, N_PIX], bf16)

            conv_w = wpack_sb[:, 0:512].rearrange("p (c k) -> p c k", c=CC)
            thwT = wpack_sb[:, 512:768].rearrange("p (c k) -> p c k", c=CC)
            WT_sb = wpack_sb[:, 768:1024]
            shift_in = vpack_sb[:, 0:2]
            thb_sb = vpack_sb[:, 2:3]
            cmi_sb = vpack_sb[:, 3:131]

            # single SP FIFO keeps the transfer order exactly as needed:
            # conv weights, b chunks (phase 1 streams them), phase-2/3
            # consts, a (only needed by phase 3), then the output stores.
            nc.sync.dma_start(out=wpack_sb[:, 0:512], in_=wpack_d[:, 0:512])
            pos = 0
            for sz in B_CHUNKS:
                off = CC * P * pos
                nc.sync.dma_start(
                    out=b_sb[:, :, pos : pos + sz],
                    in_=b_d[off : off + CC * P * sz].rearrange(
                        "(c p x) -> p c x", c=CC, p=P),
                )
                pos += sz
            assert pos == N_PIX
            nc.sync.dma_start(out=wpack_sb[:, 512:WCOLS],
                              in_=wpack_d[:, 512:WCOLS])
            nc.sync.dma_start(out=vpack_sb[:], in_=vpack_d[:])
            nc.sync.dma_start(out=a_sb[:], in_=a_d.rearrange("c p x -> p c x"))

            # ---- engine warmup ------------------------------------------
            # Touch the scalar engine immediately so its activation-table
            # load (1.3us) runs during the initial DMA wait, not in front of
            # the first phase-1 eviction.
            act_warm = cpool.tile([P, 8], f32)
            nc.scalar.memzero(act_warm[:, 0:4])
            nc.scalar.copy(act_warm[:, 4:8], act_warm[:, 0:4])

            # ---- PE warmup: garbage matmuls on the weight pack ------------
            if WARMUP_MM:
                warm_ps = ppool.tile([P, RB], f32, tag="phi", name="warm_ps")
                for i in range(WARMUP_MM):
                    nc.tensor.matmul(
                        warm_ps[:, 0:256], wpack_sb[:, 0:P],
                        wpack_sb[:, 0:256], start=True, stop=True,
                    )

            # ---- phase 1: Mi accumulation, software-pipelined by quad -----
            # quad qd = pixel chunks 4qd..4qd+3 (128 px each) = b chunk qd.
            mi_ps = ppool.tile([Ci, Ci], f32, tag="mi", bufs=1, name="mi_ps")
            q_sbs = {}

            def emit_quad(qd):
                # [pix, 2 x (phiT | gT)] lo/hi halves; fully separate PSUM
                # tiles so the DVE and ACT evictions share no dependencies.
                lo_ps = ppool.tile([P, 2, 256], f32, tag="plo", name=f"lops{qd}")
                hi_ps = ppool.tile([P, 2, 256], f32, tag="phi", name=f"hips{qd}")
                for k in range(4):
                    m = 4 * qd + k
                    dst = lo_ps if k < 2 else hi_ps
                    for cc in range(CC):
                        nc.tensor.matmul(
                            dst[:, k % 2, :],
                            b_sb[:, cc, m * P : (m + 1) * P],
                            conv_w[:, cc, :],
                            start=(cc == 0), stop=(cc == CC - 1),
                        )
                qd_lo = wpool.tile([P, 2, 256], bf16, tag="qlo", bufs=3,
                                   name=f"qlo{qd}")
                qd_hi = wpool.tile([P, 2, 256], bf16, tag="qhi", bufs=3,
                                   name=f"qhi{qd}")
                nc.vector.tensor_copy(qd_lo[:], lo_ps[:])
                nc.scalar.copy(qd_hi[:], hi_ps[:])
                q_sbs[qd] = (qd_lo, qd_hi)

            # flipped: mi_ps[ci2, ci1] = Mi[ci1, ci2] (g as lhsT, phi as
            # rhs) so R^T comes out of a single matmul later.
            def emit_mi(qd, ks=(0, 1, 2, 3)):
                for k in ks:
                    half = q_sbs[qd][k // 2]
                    kk = k % 2
                    nc.tensor.matmul(
                        mi_ps[:], half[:, kk, P:256], half[:, kk, 0:P],
                        start=(qd == 0 and k == 0),
                        stop=(qd == NCH - 1 and k == 3),
                    )

            emit_quad(0)
            for qd in range(1, NCH):
                emit_quad(qd)
                if qd < NCH - 1:
                    emit_mi(qd - 1)
            emit_mi(NCH - 2)

            # ---- tail: finish Mi while ta = theta'^T a fills the PE -------
            NBLK = HALF // RB
            ta_sb = bpool.tile([Ci, HALF], bf16)
            mi_sb = bpool.tile([Ci, Ci], bf16)
            rt_sb = bpool.tile([Ci, C], bf16)

            def emit_ta(t):
                rows = slice(t * RB, (t + 1) * RB)
                ta_ps = ppool.tile([Ci, RB], f32,
                                   tag=("plo" if t % 2 else "phi"),
                                   name=f"taps{t}")
                for cc in range(CC):
                    nc.tensor.matmul(ta_ps[:], thwT[:, cc, :],
                                     a_sb[:, cc, rows],
                                     start=(cc == 0), stop=(cc == CC - 1))
                if t < 3:
                    nc.scalar.activation(ta_sb[:, rows], ta_ps[:],
                                         Act.Identity, bias=thb_sb)
                else:
                    nc.vector.tensor_tensor(
                        ta_sb[:, rows], ta_ps[:],
                        thb_sb.broadcast_to([Ci, RB]),
                        op=mybir.AluOpType.add)

            emit_mi(NCH - 1, (0, 1))
            emit_ta(0)
            emit_ta(1)
            emit_mi(NCH - 1, (2, 3))
            nc.vector.tensor_tensor(mi_sb[:], mi_ps[:], cmi_sb,
                                    op=mybir.AluOpType.add)
            emit_ta(2)
            rt_ps = ppool.tile([Ci, C], f32, tag="phi", name="rt_ps")
            nc.tensor.matmul(rt_ps[:], mi_sb[:], WT_sb[:],
                             start=True, stop=True)
            nc.vector.tensor_copy(rt_sb[:], rt_ps[:])
            emit_ta(3)

            # ---- out = R^T-contract ta, BN shift, store -------------------
            for r in range(NBLK):
                rows = slice(r * RB, (r + 1) * RB)
                osb = wpool.tile([P, CC, RB], bf16, tag="osb", bufs=4,
                                 name=f"osb{r}")
                for co in range(CC):
                    o_ps = ppool.tile([P, RB], f32,
                                      tag=("plo" if co else "phi"),
                                      name=f"ops{r}{co}")
                    nc.tensor.matmul(o_ps[:], rt_sb[:, co * P : (co + 1) * P],
                                     ta_sb[:, rows], start=True, stop=True)
                    if co == 0:
                        nc.scalar.activation(osb[:, 0, :], o_ps[:],
                                             Act.Identity,
                                             bias=shift_in[:, 0:1])
                    else:
                        nc.vector.tensor_tensor(
                            osb[:, 1, :], o_ps[:],
                            shift_in[:, 1:2].broadcast_to([P, RB]),
                            op=mybir.AluOpType.add,
                        )
                nc.sync.dma_start(
                    out=out_d[:, :, rows].rearrange("c p r -> p c r"),
                    in_=osb[:],
                )

    nc.compile()
    return nc


def _get_nc():
    if "nc" not in _CACHE:
        _CACHE["nc"] = _build()
    return _CACHE["nc"]


def _prep_in_maps(a, b, theta_w, theta_b, phi_w, phi_b, g_w, g_b, W_w,
                  bn_gamma, bn_beta, bn_mean, bn_var):
    import ml_dtypes

    f = np.float32
    bf = ml_dtypes.bfloat16
    a4 = np.asarray(a, f).reshape(B, C, N_PIX)
    b4 = np.asarray(b, f).reshape(B, C, N_PIX)
    theta_w = np.asarray(theta_w, f)
    phi_w = np.asarray(phi_w, f)
    g_w = np.asarray(g_w, f)
    W_w = np.asarray(W_w, f)
    theta_b = np.asarray(theta_b, f)
    phi_b = np.asarray(phi_b, f)
    g_b = np.asarray(g_b, f)

    scale = (np.asarray(bn_gamma, f)
             / np.sqrt(np.asarray(bn_var, f) + BN_EPS)).astype(f)
    shift = (np.asarray(bn_beta, f) - np.asarray(bn_mean, f) * scale).astype(f)
    inv_n = 1.0 / np.float64(N_PIX)

    wpack = np.zeros((P, WCOLS), f)
    wpack[:, 0:128] = phi_w.T[0:P]
    wpack[:, 128:256] = g_w.T[0:P]
    wpack[:, 256:384] = phi_w.T[P:C]
    wpack[:, 384:512] = g_w.T[P:C]
    thT = (theta_w * inv_n).T                   # [C, Ci]
    wpack[:, 512:640] = thT[0:P]
    wpack[:, 640:768] = thT[P:C]
    wpack[:, 768:1024] = (W_w * scale[:, None]).T
    wpack = np.ascontiguousarray(wpack.astype(bf))

    # Mi bias correction from rowsums of b (exact; zero when biases are zero)
    rsb = b4.sum(axis=2)                        # [B, C]
    s_phi = rsb @ phi_w.T                       # [B, Ci]
    s_g = rsb @ g_w.T                           # [B, Ci]

    in_maps = []
    for core in range(N_CORES):
        s, h = divmod(core, 2)
        cmi = (phi_b[:, None] * s_g[s][None, :]
               + s_phi[s][:, None] * g_b[None, :]
               + N_PIX * phi_b[:, None] * g_b[None, :]).astype(f)
        vpack = np.zeros((P, VCOLS), f)
        vpack[:, 0] = shift[:P]
        vpack[:, 1] = shift[P:]
        vpack[:, 2] = theta_b * inv_n
        vpack[:, 3:131] = cmi.T
        in_maps.append({
            "a_half": np.ascontiguousarray(
                a4[s][:, h * HALF : (h + 1) * HALF]
                .reshape(CC, P, HALF).astype(bf)),
            "b_full": np.concatenate([
                np.ascontiguousarray(
                    b4[s].reshape(CC, P, N_PIX)[:, :, p0:p0 + sz]).ravel()
                for p0, sz in zip(np.cumsum((0,) + B_CHUNKS[:-1]), B_CHUNKS)
            ]).astype(bf),
            "wpack": wpack,
            "vpack": np.ascontiguousarray(vpack),
        })
    return in_maps


def run(inputs: dict, trace: bool = False):
    from concourse.bass_utils import run_bass_kernel_spmd

    nc = _get_nc()
    in_maps = _prep_in_maps(**inputs)
    res = run_bass_kernel_spmd(nc, in_maps, list(range(N_CORES)), trace=trace)
    out = np.empty((B, C, N_PIX), np.float32)
    for core in range(N_CORES):
        s, h = divmod(core, 2)
        out[s][:, h * HALF : (h + 1) * HALF] = \
            res.results[core]["out"].reshape(C, HALF).astype(np.float32)
    return out.reshape(B, C, H, W), res


def kernel(**inputs) -> np.ndarray:
    out, _ = run(inputs, trace=False)
    return out
